# revision 9
# baseline (speedup 1.0000x reference)
"""GCN aggregator kernel for Trainium2 (Bass/Tile), 8-core data-parallel.

Computes: out = relu(((sum_k neigh[:,k,:] + self) / (K+1)) @ W + b)
Sharding: nodes (N) split evenly across 8 NeuronCores; W/b replicated.

The kernel is HBM-bandwidth bound (ridge regime): per-core traffic in f32
is ~119MB against a ~410 GB/s achievable per-NC DMA rate. The rel-err
budget (2e-2) has ~4x margin over bf16 rounding (~5e-3 measured
end-to-end), so all streams are cast to bf16 on the host, halving HBM
traffic and doubling DVE throughput (2x perf mode).

Per 128-node tile on each core:
  1. DMA packed [neigh;self] tile in two 13-group halves    (sync HWDGE)
  2. DVE pairwise tree-fold of each half down to 2 partials (VectorE 2x)
  3. PE transpose-accumulates the 4 partials into PSUM,
     ACT copies chunk-wise w/ 1/(K+1) scale -> bf16 lhsT    (TensorE/ScalarE)
  4. PE GEMM sumT.T @ W (bf16) accumulated over 4 d-chunks
     + bias via ones-matmul                                 (TensorE)
  5. ACT relu PSUM->SBUF bf16, DMA store                    (ScalarE HWDGE)

DVE folds only 13->2 per half (not ->1) so VectorE stays ~25% under the
DMA tile period; the PE (mostly idle, pinned at its 1.2 GHz mid p-state
by bursty occupancy) absorbs the last fold levels as extra accumulating
transposes.

Host: inputs packed+cast to bf16 (numpy round-to-nearest via ml_dtypes);
bf16 output upcast to f32 before returning.
"""

import os
import sys

import numpy as np
import ml_dtypes

for _p in ("/opt/trn_rl_repo", "/root/.axon_site/_ro/trn_rl_repo"):
    if os.path.isdir(_p) and _p not in sys.path:
        sys.path.insert(0, _p)

import concourse.bass as bass
import concourse.tile as tile
from concourse import bacc, mybir
from concourse.masks import make_identity

N, K, D, O = 16384, 25, 512, 1024
G = K + 1  # neigh groups + self
N_CORES = 8
P = 128  # nodes per tile (partition count)
INV = 1.0 / (K + 1)
FP = mybir.dt.float32
BF = mybir.dt.bfloat16
NP_BF = ml_dtypes.bfloat16


def _fold_to(nc, t, g, T):
    """In-place pairwise fold of `g` contiguous D-sized groups in tile t
    down to `T` partial groups (left in t[:, :T*D])."""
    while g > T:
        lo = g // 2
        if g - lo < T:
            lo = g - T
        nc.vector.tensor_add(
            t[:, : lo * D], t[:, : lo * D], t[:, (g - lo) * D : g * D]
        )
        g -= lo


def build_nc(n_nodes: int, neigh_bufs: int = 6, T: int = 2) -> bass.Bass:
    """Build the per-core Bass program for a shard of `n_nodes` nodes."""
    assert n_nodes % P == 0
    nt = n_nodes // P

    nc = bacc.Bacc("TRN2", target_bir_lowering=False, debug=False)
    packed_h = nc.dram_tensor("packed", [n_nodes, G * D], BF, kind="ExternalInput")
    w_h = nc.dram_tensor("W", [D, O], BF, kind="ExternalInput")
    b_h = nc.dram_tensor("b", [O], BF, kind="ExternalInput")
    out_h = nc.dram_tensor("out", [n_nodes, O], BF, kind="ExternalOutput")

    n_dc = D // P  # d-chunks for the GEMM contraction
    g1 = G // 2  # 13 groups per half
    g2 = G - g1

    with tile.TileContext(nc) as tc:
        with (
            tc.tile_pool(name="const", bufs=1) as const_pool,
            tc.tile_pool(name="neigh", bufs=neigh_bufs) as neigh_pool,
            tc.tile_pool(name="small", bufs=3) as small_pool,
            tc.tile_pool(name="outp", bufs=3) as out_pool,
            tc.tile_pool(name="ps_t", bufs=2, space="PSUM") as ps_t_pool,
            tc.tile_pool(name="ps_o", bufs=2, space="PSUM") as ps_o_pool,
        ):
            # --- constants (w_sb/b_sb DMAs are emitted after tile 0's loads
            # below, so the neigh stream starts immediately on the ring) ---
            # w_sb[p, c, o] = W[c*128 + p, o] -> chunk c is the rhs for d-chunk c
            w_sb = const_pool.tile([P, n_dc * O], BF)
            b_sb = const_pool.tile([1, O], BF)
            ident = const_pool.tile([P, P], BF)
            make_identity(nc, ident)
            ones = const_pool.tile([1, P], BF)
            nc.gpsimd.memset(ones, 1.0)

            n_oh = O // 512

            for i in range(nt):
                # split the load in halves so the k-sum (DVE tree adds in
                # bf16 2x mode) starts while the second half streams, and
                # SBUF slots release at half-tile granularity. self_vecs is
                # pre-packed as the 26th group so no separate add is needed.
                nh1 = neigh_pool.tile([P, g1 * D], BF, tag="nh1", name="nh1")
                nc.sync.dma_start(nh1, packed_h[bass.ts(i, P), : g1 * D])
                nh2 = neigh_pool.tile([P, g2 * D], BF, tag="nh2", name="nh2")
                nc.sync.dma_start(nh2, packed_h[bass.ts(i, P), g1 * D :])
                if i == 0:
                    nc.sync.dma_start(
                        w_sb, w_h[:, :].rearrange("(c p) o -> p c o", p=P)
                    )
                    nc.sync.dma_start(b_sb, b_h[:])

                _fold_to(nc, nh1, g1, T)
                _fold_to(nc, nh2, g2, T)

                # Transpose-accumulate the 2T partials into f32 PSUM via
                # regular matmul (chunk.T @ I) -- standard accumulation
                # semantics (bf16-PSUM is_transpose accumulation does NOT
                # accumulate on TRN2, and split/interleaved accumulation
                # groups into one PSUM region get reordered -- each region's
                # group must be one contiguous run). Chunk-major so the
                # scaled chunk-copy releases the GEMM chunk by chunk.
                parts = [(nh1, j) for j in range(T)] + [(nh2, j) for j in range(T)]
                tps = ps_t_pool.tile([P, D], FP, tag="tps", name="tps")
                sumT = small_pool.tile([P, D], BF, tag="tsb", name="tsb")
                for c in range(n_dc):
                    for pi, (buf, j) in enumerate(parts):
                        nc.tensor.matmul(
                            tps[:, bass.ts(c, P)],
                            lhsT=buf[:, j * D + c * P : j * D + (c + 1) * P],
                            rhs=ident,
                            start=(pi == 0),
                            stop=(pi == len(parts) - 1),
                        )
                    nc.scalar.activation(
                        sumT[:, bass.ts(c, P)],
                        tps[:, bass.ts(c, P)],
                        mybir.ActivationFunctionType.Copy,
                        scale=INV,
                    )

                out_sb = out_pool.tile([P, O], BF)
                out_pss = [
                    ps_o_pool.tile([P, 512], FP, tag=f"out_ps{oh}", name=f"out_ps{oh}")
                    for oh in range(n_oh)
                ]
                for c in range(n_dc):
                    for oh in range(n_oh):
                        nc.tensor.matmul(
                            out_pss[oh],
                            lhsT=sumT[:, bass.ts(c, P)],
                            rhs=w_sb[:, c * O + oh * 512 : c * O + oh * 512 + 512],
                            start=(c == 0),
                            stop=False,
                        )

                for oh in range(n_oh):
                    # bias via K=1 matmul: ones.T @ b broadcasts b over nodes
                    nc.tensor.matmul(
                        out_pss[oh],
                        lhsT=ones,
                        rhs=b_sb[:, bass.ts(oh, 512)],
                        start=False,
                        stop=True,
                    )
                    nc.scalar.activation(
                        out_sb[:, bass.ts(oh, 512)],
                        out_pss[oh],
                        mybir.ActivationFunctionType.Relu,
                    )
                    if i == nt - 1:
                        # split the last tile's store per o-half to shorten
                        # the post-final-load tail
                        nc.scalar.dma_start(
                            out_h[bass.ts(i, P), bass.ts(oh, 512)],
                            out_sb[:, bass.ts(oh, 512)],
                        )
                if i != nt - 1:
                    nc.scalar.dma_start(out_h[bass.ts(i, P), :], out_sb)

    nc.compile()
    return nc


def shard_inputs(inputs: dict) -> list[dict]:
    n = inputs["self_vecs"].shape[0]
    per = n // N_CORES
    # pack [neigh ; self] into one contiguous bf16 stream: [N, G, D]
    packed = np.empty((n, G, D), dtype=NP_BF)
    packed[:, :K] = inputs["neigh_vecs"]
    packed[:, K] = inputs["self_vecs"]
    packed = packed.reshape(n, G * D)
    w_bf = np.ascontiguousarray(inputs["W"], dtype=NP_BF)
    b_bf = np.ascontiguousarray(inputs["b"], dtype=NP_BF)
    maps = []
    for c in range(N_CORES):
        sl = slice(c * per, (c + 1) * per)
        maps.append({"packed": packed[sl], "W": w_bf, "b": b_bf})
    return maps


def run_sharded(inputs: dict, trace: bool = False, **kwargs):
    from concourse.bass_utils import run_bass_kernel_spmd

    in_maps = shard_inputs(inputs)
    n_nodes = in_maps[0]["packed"].shape[0]
    nc = build_nc(n_nodes)
    res = run_bass_kernel_spmd(
        nc, in_maps, core_ids=list(range(N_CORES)), trace=trace, **kwargs
    )
    out = np.concatenate(
        [res.results[c]["out"] for c in range(N_CORES)], axis=0
    ).astype(np.float32)
    return out, res


def kernel(**inputs) -> np.ndarray:
    out, _ = run_sharded(inputs, trace=False)
    return out


# revision 10
# speedup vs baseline: 1.0473x; 1.0473x over previous
"""GCN aggregator kernel for Trainium2 (Bass/Tile), 8-core data-parallel.

Computes: out = relu(((sum_k neigh[:,k,:] + self) / (K+1)) @ W + b)
Sharding: nodes (N) split evenly across 8 NeuronCores; W/b replicated.

The kernel is HBM-bandwidth bound (ridge regime): per-core traffic in f32
is ~119MB against a ~410 GB/s achievable per-NC DMA rate. The rel-err
budget (2e-2) has ~4x margin over bf16 rounding (~5e-3 measured
end-to-end), so all streams are cast to bf16 on the host, halving HBM
traffic and doubling DVE throughput (2x perf mode).

Per 128-node tile on each core:
  1. DMA packed [neigh;self] tile in two 13-group halves    (sync HWDGE)
  2. DVE pairwise tree-fold of each half down to 2 partials (VectorE 2x)
  3. PE transpose-accumulates the 4 partials into PSUM,
     ACT copies chunk-wise w/ 1/(K+1) scale -> bf16 lhsT    (TensorE/ScalarE)
  4. PE GEMM sumT.T @ W (bf16) accumulated over 4 d-chunks
     + bias via ones-matmul                                 (TensorE)
  5. ACT relu PSUM->SBUF bf16, DMA store                    (ScalarE HWDGE)

DVE folds only 13->2 per half (not ->1) so VectorE stays ~25% under the
DMA tile period; the PE (mostly idle, pinned at its 1.2 GHz mid p-state
by bursty occupancy) absorbs the last fold levels as extra accumulating
transposes.

Host: inputs packed+cast to bf16 (numpy round-to-nearest via ml_dtypes);
bf16 output upcast to f32 before returning.
"""

import os
import sys

import numpy as np
import ml_dtypes

for _p in ("/opt/trn_rl_repo", "/root/.axon_site/_ro/trn_rl_repo"):
    if os.path.isdir(_p) and _p not in sys.path:
        sys.path.insert(0, _p)

import concourse.bass as bass
import concourse.tile as tile
from concourse import bacc, mybir
from concourse.masks import make_identity

N, K, D, O = 16384, 25, 512, 1024
G = K + 1  # neigh groups + self
N_CORES = 8
P = 128  # nodes per tile (partition count)
INV = 1.0 / (K + 1)
FP = mybir.dt.float32
BF = mybir.dt.bfloat16
NP_BF = ml_dtypes.bfloat16


def _fold_to(nc, t, g, T):
    """In-place pairwise fold of `g` contiguous D-sized groups in tile t
    down to `T` partial groups (left in t[:, :T*D])."""
    while g > T:
        lo = g // 2
        if g - lo < T:
            lo = g - T
        nc.vector.tensor_add(
            t[:, : lo * D], t[:, : lo * D], t[:, (g - lo) * D : g * D]
        )
        g -= lo


def build_nc(n_nodes: int, neigh_bufs: int = 6, T: int = 2) -> bass.Bass:
    """Build the per-core Bass program for a shard of `n_nodes` nodes."""
    assert n_nodes % P == 0
    nt = n_nodes // P

    nc = bacc.Bacc("TRN2", target_bir_lowering=False, debug=False)
    packed_h = nc.dram_tensor("packed", [n_nodes, G * D], BF, kind="ExternalInput")
    w_h = nc.dram_tensor("W", [D, O], BF, kind="ExternalInput")
    b_h = nc.dram_tensor("b", [O], BF, kind="ExternalInput")
    out_h = nc.dram_tensor("out", [n_nodes, O], BF, kind="ExternalOutput")

    n_dc = D // P  # d-chunks for the GEMM contraction
    g1 = G // 2  # 13 groups per half
    g2 = G - g1

    with tile.TileContext(nc) as tc:
        with (
            tc.tile_pool(name="const", bufs=1) as const_pool,
            tc.tile_pool(name="neigh", bufs=neigh_bufs) as neigh_pool,
            tc.tile_pool(name="small", bufs=3) as small_pool,
            tc.tile_pool(name="outp", bufs=3) as out_pool,
            tc.tile_pool(name="ps_t", bufs=2, space="PSUM") as ps_t_pool,
            tc.tile_pool(name="ps_o", bufs=2, space="PSUM") as ps_o_pool,
        ):
            # --- constants (w_sb/b_sb DMAs are emitted after tile 0's loads
            # below, so the neigh stream starts immediately on the ring) ---
            # w_sb[p, c, o] = W[c*128 + p, o] -> chunk c is the rhs for d-chunk c
            w_sb = const_pool.tile([P, n_dc * O], BF)
            b_sb = const_pool.tile([1, O], BF)
            ident = const_pool.tile([P, P], BF)
            make_identity(nc, ident)
            ones = const_pool.tile([1, P], BF)
            nc.gpsimd.memset(ones, 1.0)

            n_oh = O // 512

            for i in range(nt):
                # split the load in halves so the k-sum (DVE tree adds in
                # bf16 2x mode) starts while the second half streams, and
                # SBUF slots release at half-tile granularity. self_vecs is
                # pre-packed as the 26th group so no separate add is needed.
                nh1 = neigh_pool.tile([P, g1 * D], BF, tag="nh1", name="nh1")
                nc.sync.dma_start(nh1, packed_h[bass.ts(i, P), : g1 * D])
                nh2 = neigh_pool.tile([P, g2 * D], BF, tag="nh2", name="nh2")
                nc.sync.dma_start(nh2, packed_h[bass.ts(i, P), g1 * D :])
                if i == 0:
                    nc.sync.dma_start(
                        w_sb, w_h[:, :].rearrange("(c p) o -> p c o", p=P)
                    )
                    nc.sync.dma_start(b_sb, b_h[:])

                _fold_to(nc, nh1, g1, 1)
                _fold_to(nc, nh2, g2, 1)
                summ = small_pool.tile([P, D], BF, tag="summ", name="summ")
                nc.vector.tensor_add(summ, nh1[:, :D], nh2[:, :D])

                # PE-transpose the sum chunk-wise; scaled chunk-copies
                # (f32->bf16, 1/(K+1)) release the GEMM chunk by chunk.
                tps = ps_t_pool.tile([P, D], BF, tag="tps", name="tps")
                sumT = small_pool.tile([P, D], BF, tag="tsb", name="tsb")
                for c in range(n_dc):
                    nc.tensor.transpose(
                        tps[:, bass.ts(c, P)], summ[:, bass.ts(c, P)], ident
                    )
                    nc.scalar.activation(
                        sumT[:, bass.ts(c, P)],
                        tps[:, bass.ts(c, P)],
                        mybir.ActivationFunctionType.Copy,
                        scale=INV,
                    )

                out_sb = out_pool.tile([P, O], BF)
                out_pss = [
                    ps_o_pool.tile([P, 512], FP, tag=f"out_ps{oh}", name=f"out_ps{oh}")
                    for oh in range(n_oh)
                ]
                for c in range(n_dc):
                    for oh in range(n_oh):
                        nc.tensor.matmul(
                            out_pss[oh],
                            lhsT=sumT[:, bass.ts(c, P)],
                            rhs=w_sb[:, c * O + oh * 512 : c * O + oh * 512 + 512],
                            start=(c == 0),
                            stop=False,
                        )

                for oh in range(n_oh):
                    # bias via K=1 matmul: ones.T @ b broadcasts b over nodes
                    nc.tensor.matmul(
                        out_pss[oh],
                        lhsT=ones,
                        rhs=b_sb[:, bass.ts(oh, 512)],
                        start=False,
                        stop=True,
                    )
                    nc.scalar.activation(
                        out_sb[:, bass.ts(oh, 512)],
                        out_pss[oh],
                        mybir.ActivationFunctionType.Relu,
                    )
                    if i == nt - 1:
                        # split the last tile's store per o-half to shorten
                        # the post-final-load tail
                        nc.scalar.dma_start(
                            out_h[bass.ts(i, P), bass.ts(oh, 512)],
                            out_sb[:, bass.ts(oh, 512)],
                        )
                if i != nt - 1:
                    nc.scalar.dma_start(out_h[bass.ts(i, P), :], out_sb)

    nc.compile()
    return nc


def shard_inputs(inputs: dict) -> list[dict]:
    n = inputs["self_vecs"].shape[0]
    per = n // N_CORES
    # pack [neigh ; self] into one contiguous bf16 stream: [N, G, D]
    packed = np.empty((n, G, D), dtype=NP_BF)
    packed[:, :K] = inputs["neigh_vecs"]
    packed[:, K] = inputs["self_vecs"]
    packed = packed.reshape(n, G * D)
    w_bf = np.ascontiguousarray(inputs["W"], dtype=NP_BF)
    b_bf = np.ascontiguousarray(inputs["b"], dtype=NP_BF)
    maps = []
    for c in range(N_CORES):
        sl = slice(c * per, (c + 1) * per)
        maps.append({"packed": packed[sl], "W": w_bf, "b": b_bf})
    return maps


def run_sharded(inputs: dict, trace: bool = False, **kwargs):
    from concourse.bass_utils import run_bass_kernel_spmd

    in_maps = shard_inputs(inputs)
    n_nodes = in_maps[0]["packed"].shape[0]
    nc = build_nc(n_nodes)
    res = run_bass_kernel_spmd(
        nc, in_maps, core_ids=list(range(N_CORES)), trace=trace, **kwargs
    )
    out = np.concatenate(
        [res.results[c]["out"] for c in range(N_CORES)], axis=0
    ).astype(np.float32)
    return out, res


def kernel(**inputs) -> np.ndarray:
    out, _ = run_sharded(inputs, trace=False)
    return out
